# revision 17
# baseline (speedup 1.0000x reference)
"""Gated GQA self-attention with KV cache, tensor-parallel over heads on 8
Trainium2 NeuronCores.

Reference computation (fp32):
    q = rms_norm((x @ w_q.T).reshape(B,L,H,HD))      # per-head rms over HD
    k = rms_norm((x @ w_k.T).reshape(B,L,HKV,HD))
    v = (x @ w_v.T).reshape(B,L,HKV,HD)
    k_t/v_t = concat(cache, new) over seq -> [B,HKV,S,HD]
    o = softmax(q @ k_t.T / sqrt(HD)) @ v_t          # full (non-causal)
    o *= sigmoid(x[..., :16] @ w_gate.T)             # per-head gate
    y = o.reshape(B,L,D) @ w_out.T

Sharding: core c owns q heads {2c, 2c+1} and kv group g=c//2 (GQA groups
stay intact).  Each core computes its heads' attention plus the partial
out-projection y_c = o_c @ w_out[:, cols_c].T; the host sums the 8
partials (replaces the all-reduce).

Device-side layout trick: everything is computed feature-on-partition
("transposed"), so scores come out [s, l] and the P matrix never needs a
transpose for the p@v matmul.  The host pre-transposes x and the weights
so the device never transposes activations either.  All matmuls run in
float32r (full fp32 data, 4-byte fast-feed mode: 1 cycle/row when the
moving free dim >= 256, i.e. 4x faster than plain fp32).

Softmax is computed without max-subtraction (scores are ~N(0,1) here;
exp cannot overflow), with the denominator accumulated on the PE via
ones-matmuls over the s-partition chunks.  The q-side rms scale, the
1/sqrt(HD), the k-side rms scale, the sigmoid gate and the softmax
denominator are all folded into cheap rank-1 broadcast multiplies.
"""

from contextlib import ExitStack

import numpy as np

import concourse.bass as bass
import concourse.tile as tile
from concourse import bacc, mybir
from concourse.bass_utils import run_bass_kernel_spmd

F32R = mybir.dt.float32r
F32 = mybir.dt.float32
AF = mybir.ActivationFunctionType

B, L, D = 2, 1024, 2048
H, HKV, HD = 16, 4, 128
CACHE = 1024
BL = B * L                  # 2048
S = CACHE + L               # 2048
NCORES = 8
QH = H // NCORES            # 2 q heads per core
JC = QH * HD                # 256 out-proj contraction cols per core
EPS = 1e-6

_CACHED_NC = None


def _build_core_program():
    """One SPMD program; per-core differences are input data only."""
    nc = bacc.Bacc("TRN2", target_bir_lowering=False, debug=False)

    xt = nc.dram_tensor("xt", [D, BL], F32R, kind="ExternalInput").ap()
    wqkv = nc.dram_tensor("wqkv", [D, 4 * HD], F32R, kind="ExternalInput").ap()
    wo = nc.dram_tensor("wo", [JC, D], F32R, kind="ExternalInput").ap()
    wg = nc.dram_tensor("wg", [H, QH], F32R, kind="ExternalInput").ap()
    ckt = nc.dram_tensor("ckt", [B, HD, CACHE], F32R, kind="ExternalInput").ap()
    cv = nc.dram_tensor("cv", [B, CACHE, HD], F32R, kind="ExternalInput").ap()
    # [:, :128] identity for PE transposes, [:, 128] all-ones column
    consts_in = nc.dram_tensor("consts", [128, 129], F32R, kind="ExternalInput").ap()
    y = nc.dram_tensor("y", [BL, D], F32, kind="ExternalOutput").ap()

    NL = BL // 512           # 4 column chunks of 512 over (b, l)
    NLP = BL // 256          # 8 narrower chunks for the x stream
    ND = D // 128            # 16 contraction chunks for the projections
    NS = S // 128            # 16 s chunks per batch
    NSC = CACHE // 128       # 8 cached s chunks

    with tile.TileContext(nc) as tc, ExitStack() as ctx:
        singles = ctx.enter_context(tc.tile_pool(name="singles", bufs=1))
        xtp = ctx.enter_context(tc.tile_pool(name="xtp", bufs=2))
        # one rotating pool of [128, <=512] working tiles: exp chunks,
        # squares, bcast factors, output staging
        work = ctx.enter_context(tc.tile_pool(name="work", bufs=5))
        cachep = ctx.enter_context(tc.tile_pool(name="cachep", bufs=1))
        frp = ctx.enter_context(tc.tile_pool(name="frp", bufs=2))

        psA = ctx.enter_context(tc.tile_pool(name="psA", bufs=2, space="PSUM"))
        psO = ctx.enter_context(tc.tile_pool(name="psO", bufs=2, space="PSUM"))
        psD = ctx.enter_context(tc.tile_pool(name="psD", bufs=2, space="PSUM"))
        psF = ctx.enter_context(tc.tile_pool(name="psF", bufs=2, space="PSUM"))

        consts = singles.tile([128, 129], F32R)
        nc.sync.dma_start(out=consts, in_=consts_in)
        ident = consts[:, 0:128]                      # PE transpose helper
        ones_col = consts[:, 128:129]                 # lhsT for partition sums
        ones_row = singles.tile([1, 128], F32R)       # lhsT for partition bcast
        nc.sync.dma_start(
            out=ones_row, in_=consts_in[:, 128:129].rearrange("p o -> o p")
        )

        bias_q = singles.tile([1, 1], F32)
        nc.vector.memset(bias_q, HD * EPS)
        bias_k = singles.tile([1, 1], F32)
        nc.vector.memset(bias_k, EPS)

        wg_sb = singles.tile([H, QH], F32R)
        nc.sync.dma_start(out=wg_sb, in_=wg)
        wqkv_sb = singles.tile([128, ND, 4 * HD], F32R)
        nc.sync.dma_start(out=wqkv_sb, in_=wqkv.rearrange("(k p) j -> p k j", p=128))
        wo_sb = singles.tile([128, QH, D], F32R)
        nc.sync.dma_start(out=wo_sb, in_=wo.rearrange("(h p) m -> p h m", p=128))

        # persistent activations, feature-on-partition
        qkvt = singles.tile([128, 4, BL], F32R)       # jc: qh0, qh1, k, v
        otg = singles.tile([128, B, QH, 2, 512], F32R)  # gated attention out
        gboth = singles.tile([QH, BL], F32)           # sigmoid gates, 2 rows
        gsplit = singles.tile([1, QH * BL], F32)      # same, packed on part 0
        qs = [
            singles.tile([1, BL], F32R, tag=f"qs{i}", name=f"qs{i}")
            for i in range(QH)
        ]  # q rms scales
        ks = singles.tile([1, BL], F32R)              # k rms scale

        # ---- phase 1: projections -------------------------------------
        for lc in range(NLP):
            sl = slice(lc * 256, lc * 256 + 256)
            xtile = xtp.tile([128, ND, 256], F32R, tag="xt")
            nc.sync.dma_start(
                out=xtile, in_=xt.rearrange("(k p) l -> p k l", p=128)[:, :, sl]
            )
            for jc in range(4):
                pp = psA.tile([128, 256], F32, tag="psA")
                for kk in range(ND):
                    nc.tensor.matmul(
                        pp,
                        wqkv_sb[:, kk, jc * 128 : jc * 128 + 128],
                        xtile[:, kk, :],
                        start=(kk == 0),
                        stop=(kk == ND - 1),
                    )
                nc.scalar.copy(qkvt[:, jc, sl], pp)
                if jc < 3:  # q0, q1, k need sum over HD of the square
                    sq = work.tile([128, 256], F32R, tag="work", name=f"sq{lc}_{jc}")
                    nc.vector.tensor_mul(sq, qkvt[:, jc, sl], qkvt[:, jc, sl])
                    ssq = psD.tile([1, 256], F32, tag="psD")
                    nc.tensor.matmul(ssq, ones_col, sq, start=True, stop=True)
                    # q: 1/sqrt(ssq + HD*eps) folds in the 1/sqrt(HD) score
                    # scale; k: plain rsqrt(ssq/HD + eps).
                    row = qs[jc] if jc < QH else ks
                    scale, bias = (1.0, bias_q) if jc < QH else (1.0 / HD, bias_k)
                    nc.scalar.activation(
                        row[:, sl], ssq, AF.Sqrt, bias=bias[:], scale=scale
                    )
            gp = psF.tile([QH, 256], F32, tag="psF")
            nc.tensor.matmul(gp, wg_sb, xtile[0:H, 0, :], start=True, stop=True)
            nc.scalar.activation(gboth[:, sl], gp, AF.Sigmoid)

        # f32r tiles trip the low-precision guard but are bit-identical to
        # f32 outside the PE; silence it.
        lowp = nc.allow_low_precision(reason="float32r == float32 layout")
        ctx.enter_context(lowp)
        for row in (*qs, ks):
            nc.vector.reciprocal(row, row)
        # pack both gate rows onto partition 0 so later 1-row ops stay
        # partition-aligned
        nc.sync.dma_start(out=gsplit, in_=gboth)

        # normalize qT columns by the folded q scale (rank-1 bcast via PE)
        for h in range(QH):
            for lc in range(NL):
                sl = slice(lc * 512, lc * 512 + 512)
                bc = psF.tile([128, 512], F32, tag="psF")
                nc.tensor.matmul(bc, ones_row, qs[h][:, sl], start=True, stop=True)
                nc.vector.tensor_mul(qkvt[:, h, sl], qkvt[:, h, sl], bc)
        for lc in range(NL):
            sl = slice(lc * 512, lc * 512 + 512)
            bc = psF.tile([128, 512], F32, tag="psF")
            nc.tensor.matmul(bc, ones_row, ks[:, sl], start=True, stop=True)
            nc.vector.tensor_mul(qkvt[:, 2, sl], qkvt[:, 2, sl], bc)

        # ---- phase 2: attention ---------------------------------------
        for b in range(B):
            boff = b * L
            ck_sb = cachep.tile([128, CACHE], F32R, tag="ck")
            nc.sync.dma_start(out=ck_sb, in_=ckt[b])
            cv_sb = cachep.tile([128, NSC, HD], F32R, tag="cv")
            nc.sync.dma_start(
                out=cv_sb, in_=cv[b].rearrange("(i p) d -> p i d", p=128)
            )
            vnew = cachep.tile([128, NSC, HD], F32R, tag="vnew")
            for i in range(NSC):
                tp = psF.tile([128, 128], F32R, tag="psF")
                nc.tensor.transpose(
                    tp, qkvt[:, 3, boff + i * 128 : boff + i * 128 + 128], ident
                )
                nc.scalar.copy(vnew[:, i, :], tp)

            for h in range(QH):
                for lc2 in range(2):
                    off = boff + lc2 * 512
                    qsl = qkvt[:, h, off : off + 512]
                    den = psD.tile([1, 512], F32, tag="psD")
                    ot = psO.tile([128, 512], F32)
                    for sc in range(NS):
                        if sc < NSC:
                            kT = ck_sb[:, sc * 128 : sc * 128 + 128]
                            vx = cv_sb[:, sc, :]
                        else:
                            j = boff + (sc - NSC) * 128
                            kT = qkvt[:, 2, j : j + 128]
                            vx = vnew[:, sc - NSC, :]
                        sp = psA.tile([128, 512], F32, tag="psA")
                        nc.tensor.matmul(sp, kT, qsl, start=True, stop=True)
                        ex = work.tile(
                            [128, 512], F32R, tag="work", name=f"ex{sc}"
                        )
                        nc.scalar.activation(ex, sp, AF.Exp)
                        nc.tensor.matmul(
                            den, ones_col, ex,
                            start=(sc == 0), stop=(sc == NS - 1),
                        )
                        nc.tensor.matmul(
                            ot, vx, ex,
                            start=(sc == 0), stop=(sc == NS - 1),
                        )
                    # per-column factor gate/den, bcast over partitions
                    fr = frp.tile([1, 512], F32R)
                    nc.vector.reciprocal(fr, den)
                    nc.vector.tensor_mul(
                        fr, fr, gsplit[:, h * BL + off : h * BL + off + 512]
                    )
                    fb = psF.tile([128, 512], F32, tag="psF")
                    nc.tensor.matmul(fb, ones_row, fr, start=True, stop=True)
                    fbs = work.tile([128, 512], F32R, tag="work", name="fbs")
                    nc.scalar.copy(fbs, fb)
                    nc.vector.tensor_mul(otg[:, b, h, lc2, :], ot, fbs)

        # ---- phase 3: partial out-projection --------------------------
        for b in range(B):
            for lc2 in range(2):
                for li in range(4):
                    row0 = b * L + lc2 * 512 + li * 128
                    for mc in range(4):
                        yp = psA.tile([128, 512], F32, tag="psA")
                        for h in range(QH):
                            nc.tensor.matmul(
                                yp,
                                otg[:, b, h, lc2, li * 128 : li * 128 + 128],
                                wo_sb[:, h, mc * 512 : mc * 512 + 512],
                                start=(h == 0),
                                stop=(h == QH - 1),
                            )
                        ysb = work.tile([128, 512], F32, tag="work", name="ysb")
                        nc.scalar.copy(ysb, yp)
                        nc.sync.dma_start(
                            out=y[row0 : row0 + 128, mc * 512 : mc * 512 + 512],
                            in_=ysb,
                        )

    nc.compile()
    return nc


def _get_nc():
    global _CACHED_NC
    if _CACHED_NC is None:
        _CACHED_NC = _build_core_program()
    return _CACHED_NC


def make_in_maps(x, w_q, w_k, w_v, w_out, w_gate, cache_k, cache_v):
    xt = np.ascontiguousarray(x.reshape(BL, D).T)
    consts_np = np.concatenate(
        [np.eye(128, dtype=np.float32), np.ones((128, 1), np.float32)], axis=1
    )
    in_maps = []
    for c in range(NCORES):
        g = c // 2
        wq_c = w_q[c * JC : (c + 1) * JC]                      # [256, D]
        wk_c = w_k[g * HD : (g + 1) * HD]                      # [128, D]
        wv_c = w_v[g * HD : (g + 1) * HD]
        wqkv_c = np.ascontiguousarray(
            np.concatenate([wq_c, wk_c, wv_c], axis=0).T      # [D, 512]
        )
        wo_c = np.ascontiguousarray(w_out[:, c * JC : (c + 1) * JC].T)  # [256, D]
        wg_c = np.ascontiguousarray(w_gate[c * QH : (c + 1) * QH].T)    # [16, 2]
        ckt_c = np.ascontiguousarray(cache_k[:, g].transpose(0, 2, 1))  # [B,HD,CACHE]
        cv_c = np.ascontiguousarray(cache_v[:, g])                      # [B,CACHE,HD]
        in_maps.append(
            {
                "xt": xt,
                "wqkv": wqkv_c,
                "wo": wo_c,
                "wg": wg_c,
                "ckt": ckt_c,
                "cv": cv_c,
                "consts": consts_np,
            }
        )
    return in_maps


def kernel(x, w_q, w_k, w_v, w_out, w_gate, cache_k, cache_v, _run_kwargs=None):
    in_maps = make_in_maps(x, w_q, w_k, w_v, w_out, w_gate, cache_k, cache_v)
    nc = _get_nc()
    res = run_bass_kernel_spmd(
        nc, in_maps, core_ids=list(range(NCORES)), **(_run_kwargs or {})
    )
    acc = np.zeros((BL, D), dtype=np.float64)
    for c in range(NCORES):
        acc += res.results[c]["y"]
    out = acc.astype(np.float32).reshape(B, L, D)
    if _run_kwargs:
        kernel.last_results = res
    return out
